# revision 8
# baseline (speedup 1.0000x reference)
"""Trainium2 Bass kernel for the buggy CrossAttention module.

Semantics replicated exactly from the reference (including its bugs):
  q = x @ q_w.T + q_b                  (k is computed-but-unused -> skipped)
  v = cross @ v_w.T + v_b
  scores = q_h . v_h / 8   (per head, "k" replaced by v per source bug)
  attn = softmax(scores)
  qkv = attn @ v_h
  qkv "reshaped" (B,H,L,DH)->(B,L,E) WITHOUT transposing back (source bug):
     out_row t' = h*(L/16) + t//16 contains tokens 16g..16g+15 concatenated
  out = qkv_reshaped @ o_w.T + o_b

Distribution: batch (16) sharded over 8 cores, 2 batches/core, no collectives.

All matmuls run in bf16 (inputs rounded to bf16, fp32 PSUM accumulation).
Layout strategy:
  - x^T, w^T produced via fp32->bf16 cast-DMA (SWDGE) + XBAR transpose-DMA
  - q^T [f, t] tiles from q-proj directly feed per-head attention
  - scores computed transposed [c, t] with v^T_h stationary
  - softmax denominator + partition-broadcast via ones-matmul on the PE
  - qkv computed per r = t%16 with strided moving operand, which emits the
    buggy-reshape layout for free
  - o-proj consumes qkv^T tiles as stationaries, bias via ones-row matmul
"""

import sys

for _p in ("/opt/trn_rl_repo",):
    if _p not in sys.path:
        sys.path.append(_p)

import numpy as np

import concourse.bass as bass
import concourse.mybir as mybir
import concourse.tile as tile
from concourse import bacc
from concourse.masks import make_identity

FP32 = mybir.dt.float32
BF16 = mybir.dt.bfloat16

B_TOTAL = 16
N_CORES = 8
B_CORE = B_TOTAL // N_CORES  # 2
L = 4096
E = 1024
LC = 77
EC = 768
H = 16
DH = 64

T_BLOCK = 2048          # tokens per attention block (g-window = 128)
T_CHUNK = 512           # PSUM free-dim chunk (one fp32 bank)
G_BLOCK = T_BLOCK // 16  # 128
FT = E // 128            # 8 f-tiles
ET = E // 128            # 8 e-tiles (contraction for q/o proj)
ECT = EC // 128          # 6 e-tiles (contraction for v proj)
LT16 = L // 16           # 256 t'-columns per head


def emit(tc, out_d, x_d, cross_d, qw_d, qb_d, vw_d, vb_d, ow_d, ob_d,
         b_core=B_CORE, l_tokens=L):
    nc = tc.nc
    n_blocks = l_tokens // T_BLOCK
    lt16 = l_tokens // 16
    ident = mybir.ActivationFunctionType.Identity
    expf = mybir.ActivationFunctionType.Exp

    # ---- internal DRAM (bf16 staging for transposes) ----
    x_bf = nc.dram_tensor("x_bf", [b_core, l_tokens, E], BF16, kind="Internal")
    qw_bf = nc.dram_tensor("qw_bf", [E, E], BF16, kind="Internal")
    ow_bf = nc.dram_tensor("ow_bf", [E, E], BF16, kind="Internal")
    vw_bf = nc.dram_tensor("vw_bf", [E, EC], BF16, kind="Internal")

    nc.gpsimd.dma_start(out=qw_bf[:, :], in_=qw_d[:, :])
    nc.gpsimd.dma_start(out=ow_bf[:, :], in_=ow_d[:, :])
    nc.gpsimd.dma_start(out=vw_bf[:, :], in_=vw_d[:, :])

    import contextlib
    with contextlib.ExitStack() as ctx:
        consts = ctx.enter_context(tc.tile_pool(name="consts", bufs=1))
        identity = consts.tile([128, 128], BF16)
        make_identity(nc, identity)
        ones77 = consts.tile([LC, LC], BF16)
        nc.vector.memset(ones77, 1.0)
        ones1 = consts.tile([1, 128], BF16)
        nc.vector.memset(ones1, 1.0)
        ob_sb = consts.tile([1, E], BF16)
        nc.gpsimd.dma_start(out=ob_sb, in_=ob_d[:].rearrange("(a b) -> a b", a=1))
        qb_sb = consts.tile([128, FT], FP32)
        vb_sb = consts.tile([128, FT], FP32)
        for j in range(FT):
            nc.sync.dma_start(
                out=qb_sb[:, j:j + 1],
                in_=qb_d[128 * j:128 * (j + 1)].rearrange("(a b) -> a b", b=1))
            nc.sync.dma_start(
                out=vb_sb[:, j:j + 1],
                in_=vb_d[128 * j:128 * (j + 1)].rearrange("(a b) -> a b", b=1))

        # ---- transposed weights in SBUF (bf16) ----
        wpool = ctx.enter_context(tc.tile_pool(name="wT", bufs=1))
        qwT = []
        owT = []
        vwT = []
        for e in range(ET):
            t_q = wpool.tile([128, E], BF16, name=f"qwT{e}")
            nc.sync.dma_start(out=t_q, in_=qw_bf[:, 128 * e:128 * (e + 1)],
                              transpose=True)
            qwT.append(t_q)
            t_o = wpool.tile([128, E], BF16, name=f"owT{e}")
            nc.sync.dma_start(out=t_o, in_=ow_bf[:, 128 * e:128 * (e + 1)],
                              transpose=True)
            owT.append(t_o)
        for e in range(ECT):
            t_v = wpool.tile([128, E], BF16, name=f"vwT{e}")
            nc.sync.dma_start(out=t_v, in_=vw_bf[:, 128 * e:128 * (e + 1)],
                              transpose=True)
            vwT.append(t_v)

        # pools reused across batches
        vpool = ctx.enter_context(tc.tile_pool(name="vpool", bufs=2))
        xpool = ctx.enter_context(tc.tile_pool(name="xpool", bufs=2))
        qpool = ctx.enter_context(tc.tile_pool(name="qpool", bufs=2))
        apool = ctx.enter_context(tc.tile_pool(name="apool", bufs=2))
        kvpool = ctx.enter_context(tc.tile_pool(name="kvpool", bufs=2))
        outpool = ctx.enter_context(tc.tile_pool(name="outpool", bufs=2))
        ps_qo = ctx.enter_context(tc.tile_pool(name="ps_qo", bufs=3, space="PSUM"))
        ps_sc = ctx.enter_context(tc.tile_pool(name="ps_sc", bufs=3, space="PSUM"))
        ps_kv = ctx.enter_context(tc.tile_pool(name="ps_kv", bufs=2, space="PSUM"))

        for b in range(b_core):
            # ---------------- v projection ----------------
            cross_f32 = vpool.tile([LC, EC], FP32, name="cross_f32")
            nc.sync.dma_start(out=cross_f32, in_=cross_d[b])
            cross_bf = vpool.tile([LC, EC], BF16, name="cross_bf")
            nc.vector.tensor_copy(cross_bf, cross_f32)
            crossT = []
            for e in range(ECT):
                psx = ps_kv.tile([128, LC], BF16, tag="ps_kv")
                nc.tensor.transpose(psx, cross_bf[:, 128 * e:128 * (e + 1)],
                                    identity[:LC, :LC])
                ct = vpool.tile([128, LC], BF16, name=f"crossT{e}")
                nc.vector.tensor_copy(ct, psx)
                crossT.append(ct)
            vT = []
            for p in range(FT):
                psv = ps_sc.tile([128, LC], FP32, tag="ps_sc")
                for e in range(ECT):
                    nc.tensor.matmul(psv, lhsT=vwT[e][:, 128 * p:128 * (p + 1)],
                                     rhs=crossT[e][:, :LC],
                                     start=(e == 0), stop=(e == ECT - 1))
                vt = vpool.tile([128, LC], BF16, name=f"vT{p}")
                nc.scalar.activation(vt, psv, ident, bias=vb_sb[:, p:p + 1],
                                     scale=1.0)
                vT.append(vt)
            vh = []
            for h in range(H):
                hi = h % 2
                psh = ps_kv.tile([LC, DH], BF16, tag="ps_kv")
                nc.tensor.transpose(psh, vT[h // 2][64 * hi:64 * (hi + 1), :LC],
                                    identity[64 * hi:64 * hi + DH,
                                             64 * hi:64 * hi + DH])
                vht = vpool.tile([LC, DH], BF16, name=f"vh{h}")
                nc.vector.tensor_copy(vht, psh)
                vh.append(vht)

            # cast x for this batch (per block, SWDGE fp32->bf16)
            for blk in range(n_blocks):
                t0 = blk * T_BLOCK
                nc.gpsimd.dma_start(out=x_bf[b, t0:t0 + T_BLOCK, :],
                                    in_=x_d[b, t0:t0 + T_BLOCK, :])

            for blk in range(n_blocks):
                t0 = blk * T_BLOCK
                # ---------------- x^T for this block ----------------
                xT = []
                for e in range(ET):
                    xt = xpool.tile([128, T_BLOCK], BF16, name=f"xT{e}")
                    nc.sync.dma_start(
                        out=xt,
                        in_=x_bf[b, t0:t0 + T_BLOCK, 128 * e:128 * (e + 1)],
                        transpose=True)
                    xT.append(xt)

                for j in range(FT):
                    # ---------------- q-proj for f-tile j ----------------
                    qT_t = qpool.tile([128, T_BLOCK], BF16, name="qT_t")
                    for c in range(T_BLOCK // T_CHUNK):
                        cs = slice(c * T_CHUNK, (c + 1) * T_CHUNK)
                        psq = ps_qo.tile([128, T_CHUNK], FP32, tag="ps_qo")
                        for e in range(ET):
                            nc.tensor.matmul(
                                psq,
                                lhsT=qwT[e][:, 128 * j:128 * (j + 1)],
                                rhs=xT[e][:, cs],
                                start=(e == 0), stop=(e == ET - 1))
                        nc.scalar.activation(qT_t[:, cs], psq, ident,
                                             bias=qb_sb[:, j:j + 1], scale=1.0)

                    for hi in range(2):
                        h = 2 * j + hi
                        hs = slice(64 * hi, 64 * (hi + 1))
                        # ---------------- scores^T + exp ----------------
                        exp_t = apool.tile([LC, T_BLOCK], BF16, name="exp_t")
                        for c in range(T_BLOCK // T_CHUNK):
                            cs = slice(c * T_CHUNK, (c + 1) * T_CHUNK)
                            pss = ps_sc.tile([LC, T_CHUNK], FP32, tag="ps_sc")
                            nc.tensor.matmul(pss, lhsT=vT[j][hs, :LC],
                                             rhs=qT_t[hs, cs],
                                             start=True, stop=True)
                            nc.scalar.activation(exp_t[:, cs], pss, expf,
                                                 scale=0.125)
                        # ---------- denom broadcast + normalize ----------
                        attn_t = apool.tile([LC, T_BLOCK], BF16, name="attn_t")
                        for c in range(T_BLOCK // T_CHUNK):
                            cs = slice(c * T_CHUNK, (c + 1) * T_CHUNK)
                            psd = ps_sc.tile([LC, T_CHUNK], FP32, tag="ps_sc")
                            nc.tensor.matmul(psd, lhsT=ones77, rhs=exp_t[:, cs],
                                             start=True, stop=True)
                            nc.vector.reciprocal(psd, psd)
                            nc.vector.tensor_mul(attn_t[:, cs], exp_t[:, cs], psd)
                        # ------- qkv with buggy-reshape layout -------
                        # attn_t free dim is t-local = 16*g_local + r
                        attn_r = attn_t[:].rearrange("p (g r) -> p r g", r=16)
                        kv_sb = []
                        for r2 in range(8):
                            psk = ps_kv.tile([128, G_BLOCK], FP32, tag="ps_kv")
                            nc.tensor.matmul(psk[0:64, :], lhsT=vh[h],
                                             rhs=attn_r[:, 2 * r2, :],
                                             start=True, stop=True)
                            nc.tensor.matmul(psk[64:128, :], lhsT=vh[h],
                                             rhs=attn_r[:, 2 * r2 + 1, :],
                                             start=True, stop=True,
                                             tile_position=(0, 64))
                            kv = kvpool.tile([128, G_BLOCK], BF16,
                                             name=f"kv{r2}")
                            nc.vector.tensor_copy(kv, psk)
                            kv_sb.append(kv)
                        # ---------------- o-proj for this t'-tile ----------------
                        row0 = h * lt16 + blk * G_BLOCK
                        ou = outpool.tile([128, E], FP32, name="ou")
                        for fc in range(E // T_CHUNK):
                            fs = slice(fc * T_CHUNK, (fc + 1) * T_CHUNK)
                            pso = ps_qo.tile([128, T_CHUNK], FP32, tag="ps_qo")
                            for ep in range(ET):
                                nc.tensor.matmul(pso, lhsT=kv_sb[ep],
                                                 rhs=owT[ep][:, fs],
                                                 start=(ep == 0), stop=False)
                            nc.tensor.matmul(pso, lhsT=ones1[:, :128],
                                             rhs=ob_sb[:, fs],
                                             start=False, stop=True)
                            nc.vector.tensor_copy(ou[:, fs], pso)
                        nc.sync.dma_start(out=out_d[b, row0:row0 + 128, :],
                                          in_=ou)


def build_program(b_core=B_CORE, l_tokens=L):
    nc = bacc.Bacc(None, target_bir_lowering=False, debug=False)
    x_d = nc.dram_tensor("x", [b_core, l_tokens, E], FP32, kind="ExternalInput")
    cross_d = nc.dram_tensor("cross", [b_core, LC, EC], FP32, kind="ExternalInput")
    qw_d = nc.dram_tensor("q_w", [E, E], FP32, kind="ExternalInput")
    qb_d = nc.dram_tensor("q_b", [E], FP32, kind="ExternalInput")
    vw_d = nc.dram_tensor("v_w", [E, EC], FP32, kind="ExternalInput")
    vb_d = nc.dram_tensor("v_b", [E], FP32, kind="ExternalInput")
    ow_d = nc.dram_tensor("o_w", [E, E], FP32, kind="ExternalInput")
    ob_d = nc.dram_tensor("o_b", [E], FP32, kind="ExternalInput")
    out_d = nc.dram_tensor("out", [b_core, l_tokens, E], FP32,
                           kind="ExternalOutput")
    with tile.TileContext(nc) as tc:
        emit(tc, out_d, x_d, cross_d, qw_d, qb_d, vw_d, vb_d, ow_d, ob_d,
             b_core=b_core, l_tokens=l_tokens)
    nc.finalize()
    return nc


_PROGRAM_CACHE = {}


def kernel(**inputs):
    from concourse import bass_utils

    x = np.ascontiguousarray(np.asarray(inputs["x"], dtype=np.float32))
    cross = np.ascontiguousarray(np.asarray(inputs["cross"], dtype=np.float32))
    weights = {
        k: np.ascontiguousarray(np.asarray(inputs[k], dtype=np.float32))
        for k in ("q_w", "q_b", "v_w", "v_b", "o_w", "o_b")
    }

    if "nc" not in _PROGRAM_CACHE:
        _PROGRAM_CACHE["nc"] = build_program()
    nc = _PROGRAM_CACHE["nc"]

    in_maps = []
    for i in range(N_CORES):
        m = {"x": x[B_CORE * i:B_CORE * (i + 1)],
             "cross": cross[B_CORE * i:B_CORE * (i + 1)]}
        m.update(weights)
        in_maps.append(m)

    res = bass_utils.run_bass_kernel_spmd(nc, in_maps,
                                          core_ids=list(range(N_CORES)))
    out = np.concatenate([r["out"] for r in res.results], axis=0)
    return out
